# revision 5
# baseline (speedup 1.0000x reference)
"""Causal self-attention Trainium2 kernel v2 (restructured pipeline).

Problem: B=8, T=1024, C=768, H=12 heads, D=64. fp32 in, fp32 out (bf16 DMA).
Sharding: data-parallel over batch - core b computes batch element b.

Differences vs v1 (205us):
- Score matmuls for the two heads of a pair are emitted back-to-back to
  different PSUM banks with row groups 0-63 / 64-127, so they overlap in
  the PE array (row-tiling concurrency) -> ~2x on scores.
- av matmuls lag the score/exp chain by one c2 iteration, so they never
  wait on a freshly issued exp.
- ScalarE runs exp only during the attention passes; all PSUM drains go
  to DVE, out-projection drains to ScalarE in the exp-light phase.
- Pass order: preamble (warm + qk0 + v0..7) -> pass1 = attn(g, j2=1) for
  all pairs with qk(1..5) as PE fillers -> pass2 = attn(g, j2=0) with
  out-proj chunks 4..7 as fillers -> tail = out-proj chunks 0..3.
- Softmax normalization: denominator rows stashed at contiguous
  partitions, one reciprocal per pair-pair, PE broadcast via a [2,128]
  0/1 pattern (one N=512 matmul per (pair, j2)), one whole-pair TT.
- Input DMAs spread need-ordered across 4 issue queues; warmup matmuls
  run on a memset tile with no DMA dependency; output DMA'd as bf16.
"""

import numpy as np
import ml_dtypes

import concourse.bass as bass
import concourse.bacc as bacc
import concourse.tile as tile
from concourse import mybir
from concourse.bass_utils import run_bass_kernel_spmd

N_CORES = 8
T = 1024
C = 768
H = 12
D = 64
P = 128
NK = C // P       # 6 contraction chunks
NT = T // P       # 8 t-chunks
NPAIR = H // 2    # 6 head pairs
F32 = mybir.dt.float32
F32R = mybir.dt.float32r
BF16 = mybir.dt.bfloat16
EXP = mybir.ActivationFunctionType.Exp
COPY = mybir.ActivationFunctionType.Copy

NPBF16 = ml_dtypes.bfloat16


def build_kernel(qk_bias=False, v_bias=False, o_bias=False):
    nc = bacc.Bacc("TRN2", target_bir_lowering=False, debug=False,
                   num_devices=N_CORES)

    xT_d = nc.dram_tensor("xT", [C, T], BF16, kind="ExternalInput").ap()
    # column-reordered: pair g occupies cols [256g, 256g+256): first 128 q,
    # next 128 k
    wqk_d = nc.dram_tensor("wqk", [C, 2 * C], BF16, kind="ExternalInput").ap()
    wv_d = nc.dram_tensor("wv", [C, C], BF16, kind="ExternalInput").ap()
    wp_d = nc.dram_tensor("wp", [C, C], BF16, kind="ExternalInput").ap()
    tril_d = nc.dram_tensor("tril", [P, P], BF16, kind="ExternalInput").ap()
    ones64_d = nc.dram_tensor("ones64b", [P, D], BF16,
                              kind="ExternalInput").ap()
    if qk_bias:
        bqk_d = nc.dram_tensor("bqk_cols", [P, 2 * NPAIR], F32,
                               kind="ExternalInput").ap()
    if v_bias:
        bv_d = nc.dram_tensor("bias_v_b", [P, C], BF16,
                              kind="ExternalInput").ap()
    if o_bias:
        bo_d = nc.dram_tensor("bias_o_b", [P, C], F32,
                              kind="ExternalInput").ap()
    out_d = nc.dram_tensor("out", [T, C], BF16, kind="ExternalOutput").ap()

    with tile.TileContext(nc) as tc:
        with tc.tile_pool(name="persist", bufs=1) as pp, \
             tc.tile_pool(name="exp_sb", bufs=8) as te_pool, \
             tc.tile_pool(name="den_sb", bufs=2) as den_pool, \
             tc.tile_pool(name="out_sb", bufs=2) as ot_pool:
            aps = tc.alloc_tile_pool(name="ps", bufs=2, space="PSUM")

            xT = [pp.tile([P, T], BF16, tag=f"xT{k}", name=f"xT{k}")
                  for k in range(NK)]
            wqk = [pp.tile([P, 2 * C], BF16, tag=f"wqk{k}", name=f"wqk{k}")
                   for k in range(NK)]
            wv = [pp.tile([P, C], BF16, tag=f"wv{k}", name=f"wv{k}")
                  for k in range(NK)]
            wp = [pp.tile([P, C], BF16, tag=f"wp{k}", name=f"wp{k}")
                  for k in range(NK)]
            qT = [pp.tile([P, T], BF16, tag=f"qT{g}", name=f"qT{g}")
                  for g in range(NPAIR)]
            kT = [pp.tile([P, T], BF16, tag=f"kT{g}", name=f"kT{g}")
                  for g in range(NPAIR)]
            v_ext = [pp.tile([P, H, D + 1], BF16, tag=f"vext{i}",
                             name=f"vext{i}") for i in range(NT)]
            yT = [pp.tile([P, T], BF16, tag=f"yT{g}", name=f"yT{g}")
                  for g in range(NPAIR)]
            tril = pp.tile([P, P], BF16, tag="tril")
            ones64 = pp.tile([P, D], BF16, tag="ones64")
            warm = pp.tile([P, 512], BF16, tag="warm")

            # ---------------- input DMAs (need-ordered, 4 queues) --------
            # warm tile has no DMA dependency: memset then matmul right away
            nc.gpsimd.memset(warm[:], 0.0)
            # xT split across sync+gpsimd first (everything waits on it),
            # then wqk pair 0; small consts and v_ext memsets go AFTER the
            # urgent DMA issues (they are needed only once attention starts)
            for k in range(NK):
                (nc.sync if k % 2 == 0 else nc.gpsimd).dma_start(
                    xT[k][:], xT_d[P * k:P * (k + 1), :])
            for k in range(NK):
                (nc.sync if k % 2 == 0 else nc.gpsimd).dma_start(
                    wqk[k][:, 0:256], wqk_d[P * k:P * (k + 1), 0:256])
            nc.gpsimd.dma_start(tril[:], tril_d[:])
            nc.gpsimd.dma_start(ones64[:], ones64_d[:])
            for i in range(NT):
                nc.gpsimd.memset(v_ext[i][:, :, D:D + 1], 1.0)
            # scalar queue: wv (needed by the v units), wqk pairs 1..5
            # (needed through pass 1), then wp (needed only at pass 2)
            for k in range(NK):
                nc.scalar.dma_start(wv[k][:], wv_d[P * k:P * (k + 1), :])
            for k in range(NK):
                nc.scalar.dma_start(wqk[k][:, 256:2 * C],
                                    wqk_d[P * k:P * (k + 1), 256:2 * C])
            for k in range(NK):
                nc.scalar.dma_start(wp[k][:], wp_d[P * k:P * (k + 1), :])
            if qk_bias:
                bqk = pp.tile([P, 2 * NPAIR], F32, tag="bqk")
                nc.sync.dma_start(bqk[:], bqk_d[:])
            if v_bias:
                bv = pp.tile([P, C], BF16, tag="bv")
                nc.sync.dma_start(bv[:], bv_d[:])
            if o_bias:
                bo = pp.tile([P, C], F32, tag="bo")
                nc.sync.dma_start(bo[:], bo_d[:])

            # PE warmup on the memset tile: release the HAM clock gate while
            # the input DMAs stream
            for _ in range(18):
                wps = aps.tile([P, 512], F32, tag="small", name="warm")
                nc.tensor.matmul(wps[:], warm[:, 0:P], warm[:],
                                 start=True, stop=True)

            # pairs whose softmax normalization has been EMITTED, per j2.
            # proj units gate on this to stay deadlock-free (a matmul that
            # waits on a not-yet-emitted norm TT would wedge the PE queue).
            normed = set()

            def dummy_unit(n):
                """Keep-warm filler: dependency-free single matmuls (closed
                psum lifecycle), pumped only when real work has run dry."""
                for _ in range(n):
                    wps = aps.tile([P, 512], F32, tag="small", name="dummy")
                    nc.tensor.matmul(wps[:], warm[:, 0:P], warm[:],
                                     start=True, stop=True)
                    yield

            # ---------------- unit generators (PE-pumped) ----------------
            def qkproj_unit(g, which):
                """QK projection for pair g; which: 0 = q, 1 = k.
                Yields once per matmul; drains on DVE."""
                col0 = 256 * g + 128 * which
                dst = (qT if which == 0 else kT)[g]
                for t2 in range(2):
                    ps = aps.tile([P, 512], F32, tag="small", name="ps_qk")
                    for k in range(NK):
                        nc.tensor.matmul(
                            ps[:],
                            wqk[k][:, col0:col0 + P],
                            xT[k][:, 512 * t2:512 * (t2 + 1)],
                            start=(k == 0), stop=(k == NK - 1))
                        yield
                    d = dst[:, 512 * t2:512 * (t2 + 1)]
                    if qk_bias:
                        nc.vector.tensor_scalar_add(
                            out=d, in0=ps[:],
                            scalar1=bqk[:, 2 * g + which:2 * g + which + 1])
                    else:
                        nc.vector.tensor_copy(d, ps[:])

            def v_unit(i):
                """v projection for t-chunk i -> v_ext[i] (DVE drain)."""
                for n2, (c0, c1) in enumerate(((0, 512), (512, 768))):
                    w = c1 - c0
                    ps = aps.tile([P, 512], F32, tag="small", name="ps_v")
                    for k in range(NK):
                        nc.tensor.matmul(
                            ps[:, 0:w],
                            xT[k][:, P * i:P * (i + 1)],
                            wv[k][:, c0:c1],
                            start=(k == 0), stop=(k == NK - 1))
                        yield
                    h0, h1 = c0 // D, c1 // D
                    ps3 = ps[:, 0:w].rearrange("p (h d) -> p h d", d=D)
                    if v_bias:
                        nc.vector.tensor_add(
                            out=v_ext[i][:, h0:h1, 0:D], in0=ps3,
                            in1=bv[:, c0:c1].rearrange("p (h d) -> p h d",
                                                       d=D))
                    else:
                        nc.vector.tensor_copy(v_ext[i][:, h0:h1, 0:D], ps3)

            def proj_unit(i, glist=tuple(range(NPAIR))):
                """output projection for t-chunk i (ScalarE drain + DMA).
                glist orders the contraction to match norm completion; each
                matmul waits (yields 'blocked') until its pair's norm for
                this chunk's j2-half has been emitted."""
                j2 = i // 4
                o_t = ot_pool.tile([P, C], BF16, tag="out", name="o_t")
                for n2, (c0, c1) in enumerate(((0, 512), (512, 768))):
                    # gate BEFORE the psum alloc: a unit must never hold an
                    # open accumulation group while blocked (the pool would
                    # hand its buffer to another unit)
                    while any((g, j2) not in normed for g in glist):
                        yield 'blocked'
                    w = c1 - c0
                    ps = aps.tile([P, 512], F32, tag="small", name="ps_o")
                    for n_g, g in enumerate(glist):
                        nc.tensor.matmul(
                            ps[:, 0:w],
                            yT[g][:, P * i:P * (i + 1)],
                            wp[g][:, c0:c1],
                            start=(n_g == 0), stop=(n_g == NPAIR - 1))
                        yield
                    dst = o_t[:, c0:c1]
                    if o_bias:
                        nc.vector.tensor_add(out=dst, in0=ps[:, 0:w],
                                             in1=bo[:, c0:c1])
                    else:
                        nc.scalar.activation(out=dst, in_=ps[:, 0:w],
                                             func=COPY)
                nc.gpsimd.dma_start(out_d[P * i:P * (i + 1), :], o_t[:])

            def norm_unit(glist, j2, den_t):
                """Batched softmax normalization for two pairs: one
                reciprocal over their denominator rows, then per head a PE
                broadcast + one TT multiply into yT (pumped generator)."""
                tq0 = 512 * j2
                np_ = 64 * len(glist)
                recd = den_pool.tile([P, 512], F32, tag="recd", name="recd")
                nc.vector.reciprocal_approx_fast(out=recd[0:np_, :],
                                                 in_=den_t[0:np_, :])
                recr = den_pool.tile([P, 512], BF16, tag="recr", name="recr")
                nc.vector.tensor_copy(recr[0:np_, :], recd[0:np_, :])
                yield
                for gg, g in enumerate(glist):
                    for hh in range(2):
                        r = 32 * (2 * gg + hh)
                        # 'psy' tag, NOT 'small': blocked proj units can
                        # hold both small bufs, and the norm bc must never
                        # depend on a proj drain (deadlock)
                        bc = aps.tile([P, 512], F32, tag="psy", name="bc")
                        nc.tensor.matmul(
                            bc[0:D, :],
                            ones64[r:r + 1, :],
                            recr[r:r + 1, :],
                            start=True, stop=True,
                            tile_position=(r, 0) if r == 96 else None)
                        yield
                        dst = yT[g][D * hh:D * (hh + 1), tq0:tq0 + 512]
                        nc.vector.tensor_mul(out=dst, in0=bc[0:D, :],
                                             in1=dst)
                        yield
                    normed.add((g, j2))

            fillers = []

            def pump(n):
                done = 0
                spins = 0
                while done < n and fillers:
                    try:
                        r = next(fillers[0])
                    except StopIteration:
                        fillers.pop(0)
                        continue
                    if r == 'blocked':
                        # rotate the blocked unit to the back; stop if every
                        # unit is blocked
                        fillers.append(fillers.pop(0))
                        spins += 1
                        if spins > len(fillers):
                            break
                        continue
                    spins = 0
                    done += 1

            def flush():
                pump(1 << 30)

            # ---------------- attention ----------------
            def attn(g, j2, den_t, rbase, pump_n, drain_scalar=False):
                tq0 = 512 * j2
                n_tk = 4 * (j2 + 1)
                ps_y = [aps.tile([D + 1, 512], F32, tag="psy",
                                 name="ps_y") for _ in range(2)]
                pend = None

                def emit_av(c2, t_e, offs):
                    for hh in range(2):
                        h = 2 * g + hh
                        for s in range(2):
                            c = 2 * c2 + s
                            off = offs[s]
                            nc.tensor.matmul(
                                ps_y[hh][:, off:512],
                                v_ext[c][:, h, :],
                                t_e[hh][:, 512 * s + off:512 * (s + 1)],
                                start=(c2 == 0 and s == 0),
                                stop=(c == n_tk - 1))

                for c2 in range(n_tk // 2):
                    offs = [max(0, P * (2 * c2 + s) - tq0) for s in range(2)]
                    # scores: both heads back-to-back for array concurrency
                    t_s = [aps.tile([P, 1024], F32, tag="big", name="ps_s")
                           for _ in range(2)]
                    for s in range(2):
                        c = 2 * c2 + s
                        off = offs[s]
                        for hh in range(2):
                            nc.tensor.matmul(
                                t_s[hh][:, 512 * s + off:512 * (s + 1)],
                                kT[g][D * hh:D * (hh + 1), P * c:P * (c + 1)],
                                qT[g][D * hh:D * (hh + 1),
                                      tq0 + off:tq0 + 512],
                                start=True, stop=True)
                    # exp + mask
                    t_e = []
                    o0 = offs[0]
                    for hh in range(2):
                        te = te_pool.tile([P, 1024], BF16, tag="exp",
                                          name="t_e")
                        t_e.append(te)
                        nc.scalar.activation(
                            out=te[:, o0:1024], in_=t_s[hh][:, o0:1024],
                            func=EXP, scale=0.125)
                        if offs[1] > 0:
                            sl = bass.AP(
                                tensor=te.tensor,
                                offset=te.offset + o0,
                                ap=[te.ap[0], [512 + P, 2], [1, P]])
                            trb = bass.AP(
                                tensor=tril.tensor,
                                offset=tril.offset,
                                ap=[tril.ap[0], [0, 2], [1, P]])
                            nc.vector.tensor_mul(out=sl, in0=sl, in1=trb)
                    # av for the previous c2 (its exp finished an
                    # iteration ago)
                    if pend is not None:
                        emit_av(*pend)
                    pump(pump_n)
                    pend = (c2, t_e, offs)
                pump(4)
                emit_av(*pend)
                # drain yT (unnormalized); denominator rows at 32-aligned
                # partitions of den_t
                for hh in range(2):
                    dst = yT[g][D * hh:D * (hh + 1), tq0:tq0 + 512]
                    r = rbase + 32 * hh
                    nc.vector.tensor_copy(dst, ps_y[hh][0:D, :])
                    nc.vector.tensor_copy(den_t[r:r + 1, :],
                                          ps_y[hh][D:D + 1, :])

            # ---------------- schedule ----------------
            # preamble: qk pair 0 + all v chunks + qk pair 1 (PE-dense,
            # ScalarE idle)
            for it in qkproj_unit(0, 0):
                pass
            for it in qkproj_unit(0, 1):
                pass
            for i in range(NT):
                for it in v_unit(i):
                    pass
            fillers.append(qkproj_unit(1, 0))
            fillers.append(qkproj_unit(1, 1))

            # pass 1: j2=1 for all pairs; fillers = qk pairs 1..5
            den_t = None
            for g in range(NPAIR):
                if g % 2 == 0:
                    den_t = den_pool.tile([P, 512], F32, tag="den",
                                          name="den")
                attn(g, 1, den_t, rbase=64 * (g % 2), pump_n=5)
                if g % 2 == 1:
                    fillers.append(norm_unit([g - 1, g], 1, den_t))
                # qk for pair g+1 must be complete before attn(g+1, ...)
                flush()
                if g + 2 < NPAIR:
                    fillers.append(qkproj_unit(g + 2, 0))
                    fillers.append(qkproj_unit(g + 2, 1))
                else:
                    fillers.append(dummy_unit(12))

            # pass 2: j2=0, pairs in reverse order so pairs (0,1) are
            # normalized last and the tail contracts them last; fillers =
            # out-proj chunks 4..7
            for i in range(4, NT):
                fillers.append(proj_unit(i))
            for g in reversed(range(NPAIR)):
                if g % 2 == 1:
                    den_t = den_pool.tile([P, 512], F32, tag="den",
                                          name="den")
                attn(g, 0, den_t, rbase=64 * (g % 2), pump_n=5)
                if g % 2 == 0:
                    fillers.append(norm_unit([g, g + 1], 0, den_t))
                if g <= 1:
                    fillers.append(dummy_unit(12))
            # hold the clock gate open while the last norm chain resolves
            # and the tail out-projections unblock
            fillers.append(dummy_unit(10))
            flush()

            # tail: out-proj chunks 0..3, contraction in norm-completion
            # order so the first matmuls never wait on the last norms
            for i in range(4):
                for it in proj_unit(i, glist=(4, 5, 2, 3, 0, 1)):
                    pass
            aps.release()

    nc.compile()
    return nc


_NC_CACHE = {}


def _get_nc(qk_bias, v_bias, o_bias):
    key = (qk_bias, v_bias, o_bias)
    if key not in _NC_CACHE:
        _NC_CACHE[key] = build_kernel(*key)
    return _NC_CACHE[key]


def make_in_maps(x, w_attn, b_attn, w_proj, b_proj, qk_bias, v_bias, o_bias):
    # reorder qk weight columns: pair g -> [q cols | k cols]
    wqk_re = np.empty((C, 2 * C), dtype=np.float32)
    for g in range(NPAIR):
        wqk_re[:, 256 * g:256 * g + 128] = w_attn[:, 128 * g:128 * (g + 1)]
        wqk_re[:, 256 * g + 128:256 * (g + 1)] = \
            w_attn[:, C + 128 * g:C + 128 * (g + 1)]
    # tril[tk, l] = 1 iff l >= tk (keep query-pos >= key-pos)
    tril = np.triu(np.ones((P, P), dtype=np.float32))

    shared = {
        "wqk": wqk_re.astype(NPBF16),
        "wv": np.ascontiguousarray(w_attn[:, 2 * C:]).astype(NPBF16),
        "wp": np.ascontiguousarray(w_proj).astype(NPBF16),
        "tril": tril.astype(NPBF16),
        "ones64b": np.ones((P, D), dtype=NPBF16),
    }
    if qk_bias:
        bq = np.empty((P, 2 * NPAIR), dtype=np.float32)
        for g in range(NPAIR):
            bq[:, 2 * g] = b_attn[128 * g:128 * (g + 1)]
            bq[:, 2 * g + 1] = b_attn[C + 128 * g:C + 128 * (g + 1)]
        shared["bqk_cols"] = bq
    if v_bias:
        shared["bias_v_b"] = np.broadcast_to(
            b_attn[2 * C:], (P, C)).astype(NPBF16)
    if o_bias:
        shared["bias_o_b"] = np.ascontiguousarray(
            np.broadcast_to(b_proj, (P, C)).astype(np.float32))
    in_maps = []
    for b in range(N_CORES):
        m = dict(shared)
        m["xT"] = np.ascontiguousarray(x[b].T).astype(NPBF16)
        in_maps.append(m)
    return in_maps


def run(x, w_attn, b_attn, w_proj, b_proj, **spmd_kwargs):
    x = np.asarray(x, dtype=np.float32)
    w_attn = np.asarray(w_attn, dtype=np.float32)
    b_attn = np.asarray(b_attn, dtype=np.float32)
    w_proj = np.asarray(w_proj, dtype=np.float32)
    b_proj = np.asarray(b_proj, dtype=np.float32)
    qk_bias = bool(np.any(b_attn[:2 * C]))
    v_bias = bool(np.any(b_attn[2 * C:]))
    o_bias = bool(np.any(b_proj))
    nc = _get_nc(qk_bias, v_bias, o_bias)
    in_maps = make_in_maps(x, w_attn, b_attn, w_proj, b_proj,
                           qk_bias, v_bias, o_bias)
    res = run_bass_kernel_spmd(nc, in_maps, core_ids=list(range(N_CORES)),
                               **spmd_kwargs)
    out = np.stack([np.asarray(res.results[b]["out"])
                    for b in range(N_CORES)], axis=0)
    return out.astype(np.float32), res


def kernel(x, w_attn, b_attn, w_proj, b_proj):
    out, _ = run(x, w_attn, b_attn, w_proj, b_proj)
    return out
